# revision 1
# baseline (speedup 1.0000x reference)
"""Trainium2 Bass kernel for CNN+GCN+MLP (nn_CNNGCN_18236431139458).

Strategy (8 NeuronCores, one chip):
  - Conv + both GCN layers: data-parallel over batch (4 samples/core).
    The scatter-aggregate is a dense matmul against the normalized
    adjacency A^T (built host-side from edge_index). A^T streams from
    HBM once per layer (samples innermost), overlapped with PE.
    Layer-1 aggregation runs in fp8 (DoubleRow, 2x PE rate) — its
    quantization noise is coherently averaged away by the layer-2
    aggregation (A >= 0, post-relu h >= 0). Layer-2 stays bf16 since
    nothing downstream washes its noise out.
  - MLP: W1 (262144 x 100) is sharded over rows (nodes) across cores.
    An on-device AllToAll reshards the GCN output from batch-sharded to
    node-sharded; each core computes a partial [32, 100] with its W1
    shard; a ReduceScatter sums partials and hands each core its own 4
    samples for the tiny MLP tail.
  - All other matmuls bf16 with f32 PSUM accumulation; tail in f32.

Layouts (per core):
  xT   [128 ic, 4 s, 2050]   feature-major input slices
  h*T  [128 f, 4 s, 2048 n]  feature-major activations
  hw   [128 n, 16 nch, 4*128] node-major GCN linear outputs
  at   [16 sc, 128 p, 2048 dst] = A^T[sc*128+p, dst]  (streamed rhs)
  w1s  [128 f, 256 n, 100]   W1 row-shard (this core's 256 nodes)
"""

import numpy as np
import ml_dtypes

import concourse.bass as bass
import concourse.mybir as mybir
import concourse.tile as tile
from concourse.tile import add_dep_helper
from concourse import bacc
from concourse.bass_utils import run_bass_kernel_spmd

BF16 = mybir.dt.bfloat16
FP8 = mybir.dt.float8e4
F32 = mybir.dt.float32
NP_BF16 = ml_dtypes.bfloat16
NP_FP8 = mybir.dt.np(FP8)

B, H, E = 32, 2050, 128
N = 2048
C = 128
G1 = G2 = 128
MLPD = 100
KS = 3
NE = 32768
NCORES = 8
BL = B // NCORES          # 4 samples per core
NSH = N // NCORES         # 256 nodes per core (W1 row shard)
RG = [list(range(NCORES))]

Relu = mybir.ActivationFunctionType.Relu
DoubleRow = mybir.MatmulPerfMode.DoubleRow


def _emit_front(nc, tc, pools, tensors):
    """conv + GCN1 + GCN2 -> h2T [128 f, 4 s, 2048 n] bf16."""
    acts, psum, apool = pools["acts"], pools["psum"], pools["apool"]
    xT_sb = tensors["xT_sb"]
    wc_sb = tensors["wc_sb"]
    cb_sb = tensors["cb_sb"]

    # ---- conv: h0T[oc, n] = relu(sum_k WcT_k.T @ xT[:, n+k] + cb) ----
    h0T = acts.tile([128, BL, N], BF16, tag="hT", bufs=2, name="h0T")
    for nt in range(4):
        for s in range(BL):
            ps = psum.tile([128, 512], F32, tag="ps", name="ps_conv")
            for k in range(KS):
                nc.tensor.matmul(
                    ps[:],
                    lhsT=wc_sb[:, k, :],
                    rhs=xT_sb[:, s, nt * 512 + k : nt * 512 + k + 512],
                    start=(k == 0),
                    stop=(k == KS - 1),
                )
            act = nc.scalar.activation(h0T[:, s, nt * 512 : (nt + 1) * 512], ps[:], Relu, bias=cb_sb[:])
            if s == 0 and nt == 0:
                tensors["anchor_conv0"] = act

    # ---- GCN layer 1: linear + fp8 DoubleRow aggregation (A8 streamed) ----
    hw1 = acts.tile([128, 16, BL * 128], FP8, tag="hw8", bufs=1, name="hw1")
    for nch in range(16):
        ps = psum.tile([128, 512], F32, tag="ps", name="ps_lin1")
        for s in range(BL):
            nc.tensor.matmul(
                ps[:, s * 128 : (s + 1) * 128],
                lhsT=h0T[:, s, nch * 128 : (nch + 1) * 128],
                rhs=tensors["gw1_sb"][:],
                start=True,
                stop=True,
            )
        nc.vector.tensor_copy(hw1[:, nch, :], ps[:])

    h1T = acts.tile([128, BL, N], BF16, tag="hT", bufs=2, name="h1T")
    at8 = tensors["at8"]
    for dt in range(4):
        pss = [psum.tile([128, 512], F32, tag="ps", name=f"ps_agg{s}")
               for s in range(BL)]
        for sc2 in range(8):
            atile = apool.tile([128, 2, 512], FP8, tag="atile", name="atile")
            nc.sync.dma_start(
                atile[:],
                at8[2 * sc2 : 2 * sc2 + 2, :, dt * 512 : (dt + 1) * 512]
                .rearrange("c p d -> p c d"),
            )
            for s in range(BL):
                nc.tensor.matmul(
                    pss[s][:],
                    lhsT=hw1[:, 2 * sc2 : 2 * sc2 + 2, s * 128 : (s + 1) * 128],
                    rhs=atile[:],
                    start=(sc2 == 0),
                    stop=(sc2 == 7),
                    perf_mode=DoubleRow,
                )
        for s in range(BL):
            act = nc.scalar.activation(h1T[:, s, dt * 512 : (dt + 1) * 512],
                                       pss[s][:], Relu, bias=tensors["gb1_sb"][:])
            tensors["anchor_agg1_end"] = act

    # ---- GCN layer 2: linear + bf16 aggregation (A resident in SBUF),
    #      sample-outer so each sample's h2T finishes early for the A2A ----
    hw2 = acts.tile([128, 16, BL * 128], BF16, tag="hw2", bufs=1, name="hw2")
    for nch in range(16):
        ps = psum.tile([128, 512], F32, tag="ps", name="ps_lin2")
        for s in range(BL):
            nc.tensor.matmul(
                ps[:, s * 128 : (s + 1) * 128],
                lhsT=h1T[:, s, nch * 128 : (nch + 1) * 128],
                rhs=tensors["gw2_sb"][:],
                start=True,
                stop=True,
            )
        nc.vector.tensor_copy(hw2[:, nch, :], ps[:])

    A_sb = tensors["A_sb"]
    # per-sample tiles so each sample's A2A staging only depends on its own
    # aggregation output
    h2Ts = []
    for s in range(BL):
        h2T_s = acts.tile([128, N], BF16, tag=f"h2T{s}", name=f"h2T{s}")
        pss = [psum.tile([128, 512], F32, tag="ps", name=f"ps_agg2_{dt}")
               for dt in range(4)]
        for sc in range(16):
            for dt in range(4):
                nc.tensor.matmul(
                    pss[dt][:],
                    lhsT=hw2[:, sc, s * 128 : (s + 1) * 128],
                    rhs=A_sb[:, sc, dt * 512 : (dt + 1) * 512],
                    start=(sc == 0),
                    stop=(sc == 15),
                )
        for dt in range(4):
            nc.scalar.activation(h2T_s[:, dt * 512 : (dt + 1) * 512],
                                 pss[dt][:], Relu, bias=tensors["gb2_sb"][:])
        h2Ts.append(h2T_s)

    return h2Ts


def _emit_tail(nc, tc, pools, tensors, h2Ts, out_ap, collectives=True):
    """AllToAll reshard + sharded MLP + ReduceScatter + local MLP tail."""
    acts, psum, psum2, dram, small, wpool = (
        pools["acts"], pools["psum"], pools["psum2"], pools["dram"],
        pools["small"], pools["wpool"],
    )
    w1s = tensors["w1s"]

    # One AllToAll per local sample, issued as soon as that sample's h2T
    # rows are done — staging + wire hide under the next sample's agg2.
    h2a = acts.tile([128, B, NSH], BF16, tag="h2a", name="h2a")
    for s in range(BL):
        a2a_in = dram.tile([NCORES, 128, NSH], BF16, tag=f"a2a_in{s}",
                           name=f"a2a_in{s}")
        a2a_out = dram.tile([NCORES, 128, NSH], BF16, tag=f"a2a_out{s}",
                            name=f"a2a_out{s}")
        for j in range(NCORES):
            nc.sync.dma_start(a2a_in[j], h2Ts[s][:, j * NSH : (j + 1) * NSH])
        if collectives:
            nc.gpsimd.collective_compute(
                "AllToAll", mybir.AluOpType.bypass, replica_groups=RG,
                ins=[a2a_in.opt()], outs=[a2a_out.opt()],
            )
            for i in range(NCORES):
                nc.sync.dma_start(h2a[:, i * BL + s, :], a2a_out[i])
        else:
            # timing stand-in: skip the wire, read staged data directly
            # (the real collective's cost is measured separately)
            for i in range(NCORES):
                nc.sync.dma_start(h2a[:, i * BL + s, :], a2a_in[i])

    # PE warmers: keep the tensor engine busy across the A2A wait so the
    # MLP matmuls run at full (ramped) clock. Results are garbage but kept
    # live via a DMA side effect.
    n_warm = tensors.get("n_warm", 20)
    if n_warm > 0:
        warm_ps = psum.tile([128, 512], F32, tag="ps", name="warm_ps")
        for w in range(n_warm):
            nc.tensor.matmul(
                warm_ps[:],
                lhsT=tensors["gw2_sb"][:],
                rhs=tensors["A_sb"][:, 0, 0:512],
                start=(w == 0),
                stop=(w == n_warm - 1),
            )
        warm_sb = small.tile([128, 1], F32, tag="warm_sb", name="warm_sb")
        nc.vector.tensor_copy(warm_sb[:], warm_ps[:, 0:1])
        warm_dr = dram.tile([128, 1], F32, tag="warm_dr", name="warm_dr")
        nc.sync.dma_start(warm_dr[:], warm_sb[:])

    # z[b, c] = sum_n  h2a[:, :, n].T @ w1s[:, n, :]   (256 k-tiles).
    # M=32 wastes 3/4 of the PE columns, so column-tile: 4 nodes run
    # concurrently in disjoint 32-col groups (tile_position), each
    # accumulating its node-subset into its own partition range.
    ps_z = psum2.tile([128, MLPD], F32, tag="psz", name="ps_z")
    WCH = 4          # stream W1 shard in 4 chunks of 64 nodes
    for ch in range(WCH):
        w1c = wpool.tile([128, NSH // WCH, MLPD], BF16, tag="w1c", bufs=2, name="w1c")
        w1c_dma = nc.sync.dma_start(w1c[:], w1s[:, ch * (NSH // WCH) : (ch + 1) * (NSH // WCH), :])
        if "anchor_agg1_end" in tensors:
            add_dep_helper(w1c_dma.ins, tensors["anchor_agg1_end"].ins,
                           reason="delay W1 prefetch past agg1's A8 stream")
        for nl in range(NSH // WCH):
            n = ch * (NSH // WCH) + nl
            j = n % 4
            nc.tensor.matmul(
                ps_z[32 * j : 32 * (j + 1), :],
                lhsT=h2a[:, :, n],
                rhs=w1c[:, nl, :],
                start=(n < 4),
                stop=(n >= NSH - 4),
                tile_position=(0, 32 * j),
            )

    z_sb = small.tile([32, MLPD], F32, tag="z_sb", name="z_sb")
    nc.vector.tensor_copy(z_sb[:], ps_z[0:32, :])
    for j in range(1, 4):
        nc.vector.tensor_add(z_sb[:], z_sb[:], ps_z[32 * j : 32 * (j + 1), :])
    rs_in = dram.tile([32, MLPD], F32, tag="rs_in", name="rs_in")
    rs_out = dram.tile([BL, MLPD], F32, tag="rs_out", name="rs_out")
    nc.sync.dma_start(rs_in[:], z_sb[:])
    zloc = small.tile([BL, MLPD], F32, tag="zloc", name="zloc")
    if collectives:
        nc.gpsimd.collective_compute(
            "ReduceScatter", mybir.AluOpType.add, replica_groups=RG,
            ins=[rs_in.opt()], outs=[rs_out.opt()],
        )
        nc.sync.dma_start(zloc[:], rs_out[:])
    else:
        nc.sync.dma_start(zloc[:], rs_in[0:BL, :])
    hm = small.tile([BL, MLPD], F32, tag="hm", name="hm")
    nc.vector.tensor_add(hm[:], zloc[:], tensors["b1r_sb"][:])
    nc.vector.tensor_scalar_max(hm[:], hm[:], 0.0)
    nc.vector.tensor_mul(hm[:], hm[:], tensors["w2r_sb"][:])
    osb = small.tile([BL, 1], F32, tag="osb", name="osb")
    nc.vector.reduce_sum(osb[:], hm[:], axis=mybir.AxisListType.X)
    nc.vector.tensor_add(osb[:], osb[:], tensors["b2r_sb"][:])
    nc.sync.dma_start(out_ap[:], osb[:])


def build_nc(front_reps=1, tail_reps=1, collectives=True, num_devices=NCORES,
             loop_all_reps=1, n_warm=0):
    """Build + compile the SPMD program. Reps>1 variants are for timing.

    loop_all_reps>1 wraps front+tail in a hardware loop with collectives
    replaced by equal-volume DMA stand-ins (collectives can't sit inside
    control flow) — used to measure whole-kernel steady-state time.
    """
    nc = bacc.Bacc("TRN2", target_bir_lowering=False, debug=False,
                   num_devices=num_devices)

    d_xT = nc.dram_tensor("xT", [BL, 128, H], BF16, kind="ExternalInput").ap()
    d_at = nc.dram_tensor("at", [16, 128, N], BF16, kind="ExternalInput").ap()
    d_at8 = nc.dram_tensor("at8", [16, 128, N], FP8, kind="ExternalInput").ap()
    d_wc = nc.dram_tensor("wc", [KS, 128, 128], BF16, kind="ExternalInput").ap()
    d_cb = nc.dram_tensor("cb", [128, 1], F32, kind="ExternalInput").ap()
    d_gw1 = nc.dram_tensor("gw1", [128, 128], BF16, kind="ExternalInput").ap()
    d_gb1 = nc.dram_tensor("gb1", [128, 1], F32, kind="ExternalInput").ap()
    d_gw2 = nc.dram_tensor("gw2", [128, 128], BF16, kind="ExternalInput").ap()
    d_gb2 = nc.dram_tensor("gb2", [128, 1], F32, kind="ExternalInput").ap()
    d_w1s = nc.dram_tensor("w1s", [128, NSH, MLPD], BF16, kind="ExternalInput").ap()
    d_b1r = nc.dram_tensor("b1r", [BL, MLPD], F32, kind="ExternalInput").ap()
    d_w2r = nc.dram_tensor("w2r", [BL, MLPD], F32, kind="ExternalInput").ap()
    d_b2r = nc.dram_tensor("b2r", [BL, 1], F32, kind="ExternalInput").ap()
    d_out = nc.dram_tensor("out", [BL, 1], F32, kind="ExternalOutput").ap()

    with tile.TileContext(nc) as tc:
        with (
            tc.tile_pool(name="const", bufs=1) as const,
            tc.tile_pool(name="acts", bufs=1) as acts,
            tc.tile_pool(name="apool", bufs=10) as apool,
            tc.tile_pool(name="wpool", bufs=2) as wpool,
            tc.tile_pool(name="small", bufs=1) as small,
            tc.tile_pool(name="psum", bufs=7, space="PSUM") as psum,
            tc.tile_pool(name="psum2", bufs=1, space="PSUM") as psum2,
            tc.tile_pool(name="dram", bufs=1, space="DRAM") as dram,
        ):
            pools = dict(const=const, acts=acts, apool=apool, wpool=wpool,
                         small=small, psum=psum, psum2=psum2, dram=dram)

            # ---- load inputs to SBUF (per-sample x slices so conv can start early) ----
            xT_sb = const.tile([128, BL, H], BF16, name="xT_sb")
            for s in range(BL):
                nc.sync.dma_start(xT_sb[:, s, :], d_xT[s])
            wc_sb = const.tile([128, KS, 128], BF16, name="wc_sb")
            nc.sync.dma_start(wc_sb[:], d_wc.rearrange("k p o -> p k o"))
            cb_sb = const.tile([128, 1], F32, name="cb_sb")
            nc.sync.dma_start(cb_sb[:], d_cb[:])
            gw1_sb = const.tile([128, 128], BF16, name="gw1_sb")
            nc.sync.dma_start(gw1_sb[:], d_gw1[:])
            gb1_sb = const.tile([128, 1], F32, name="gb1_sb")
            nc.sync.dma_start(gb1_sb[:], d_gb1[:])
            gw2_sb = const.tile([128, 128], BF16, name="gw2_sb")
            nc.sync.dma_start(gw2_sb[:], d_gw2[:])
            gb2_sb = const.tile([128, 1], F32, name="gb2_sb")
            nc.sync.dma_start(gb2_sb[:], d_gb2[:])
            # bf16 A^T resident for the sample-outer layer-2 aggregation;
            # load overlaps conv + layer 1
            A_sb = const.tile([128, 16, N], BF16, name="A_sb")
            a_sb_dmas = []
            for q in range(4):
                a_sb_dmas.append(nc.sync.dma_start(
                    A_sb[:, 4 * q : 4 * q + 4, :],
                    d_at[4 * q : 4 * q + 4].rearrange("c p d -> p c d"),
                ))
            b1r_sb = small.tile([BL, MLPD], F32, name="b1r_sb")
            nc.sync.dma_start(b1r_sb[:], d_b1r[:])
            w2r_sb = small.tile([BL, MLPD], F32, name="w2r_sb")
            nc.sync.dma_start(w2r_sb[:], d_w2r[:])
            b2r_sb = small.tile([BL, 1], F32, name="b2r_sb")
            nc.sync.dma_start(b2r_sb[:], d_b2r[:])

            tensors = dict(
                xT_sb=xT_sb, wc_sb=wc_sb, cb_sb=cb_sb, at=d_at, at8=d_at8,
                A_sb=A_sb,
                gw1_sb=gw1_sb, gb1_sb=gb1_sb, gw2_sb=gw2_sb, gb2_sb=gb2_sb,
                w1s=d_w1s, b1r_sb=b1r_sb, w2r_sb=w2r_sb, b2r_sb=b2r_sb,
                n_warm=n_warm,
            )

            if loop_all_reps > 1:
                with tc.For_i(0, loop_all_reps, 1,
                              hint_engines=(mybir.EngineType.PE,)):
                    h2Ts = _emit_front(nc, tc, pools, tensors)
                    _emit_tail(nc, tc, pools, tensors, h2Ts, d_out,
                               collectives=False)
            elif front_reps == 1:
                h2Ts = _emit_front(nc, tc, pools, tensors)
                if "anchor_conv0" in tensors:
                    for d in a_sb_dmas:
                        add_dep_helper(d.ins, tensors["anchor_conv0"].ins,
                                       reason="delay A load past x load + conv start")
                for _ in range(tail_reps):
                    _emit_tail(nc, tc, pools, tensors, h2Ts, d_out,
                               collectives=collectives)
            else:
                with tc.For_i(0, front_reps, 1,
                              hint_engines=(mybir.EngineType.PE,)):
                    h2Ts = _emit_front(nc, tc, pools, tensors)
                for _ in range(tail_reps):
                    _emit_tail(nc, tc, pools, tensors, h2Ts, d_out,
                               collectives=collectives)

    nc.compile()
    return nc


def _prep_inputs(x, edge_index, conv_w, conv_b, gW1, gb1, gW2, gb2, W1, b1, W2, b2):
    """Host-side sharding / layout prep -> per-core input maps."""
    # gcn_norm (add_self_loops=True), duplicated edges accumulate
    src = np.concatenate([np.asarray(edge_index[0]), np.arange(N, dtype=np.int64)])
    dst = np.concatenate([np.asarray(edge_index[1]), np.arange(N, dtype=np.int64)])
    deg = np.bincount(dst, minlength=N).astype(np.float32)
    dinv = (1.0 / np.sqrt(np.maximum(deg, 1.0))).astype(np.float32)
    normv = dinv[src] * dinv[dst]
    AT = np.zeros((N, N), np.float32)
    np.add.at(AT, (src, dst), normv)
    at3 = np.ascontiguousarray(AT.reshape(16, 128, N))
    at_tiled = at3.astype(NP_BF16)
    at8_tiled = at3.astype(NP_FP8)

    wc = np.ascontiguousarray(
        np.asarray(conv_w)[:, 0, :, :].transpose(1, 2, 0)  # [KS, ic, oc]
    ).astype(NP_BF16)
    cb = np.asarray(conv_b, np.float32).reshape(128, 1)
    gw1 = np.asarray(gW1).astype(NP_BF16)
    gb1_ = np.asarray(gb1, np.float32).reshape(128, 1)
    gw2 = np.asarray(gW2).astype(NP_BF16)
    gb2_ = np.asarray(gb2, np.float32).reshape(128, 1)
    b1r = np.ascontiguousarray(np.broadcast_to(np.asarray(b1, np.float32), (BL, MLPD)))
    w2r = np.ascontiguousarray(np.broadcast_to(np.asarray(W2, np.float32)[:, 0], (BL, MLPD)))
    b2r = np.full((BL, 1), np.asarray(b2, np.float32)[0], np.float32)

    W1r = np.asarray(W1, np.float32).reshape(N, G2, MLPD)
    x_np = np.asarray(x, np.float32)

    in_maps = []
    for c in range(NCORES):
        xT = np.ascontiguousarray(
            x_np[c * BL : (c + 1) * BL].transpose(0, 2, 1)
        ).astype(NP_BF16)
        w1s = np.ascontiguousarray(
            W1r[c * NSH : (c + 1) * NSH].transpose(1, 0, 2)
        ).astype(NP_BF16)
        in_maps.append({
            "xT": xT, "at": at_tiled, "at8": at8_tiled, "wc": wc, "cb": cb,
            "gw1": gw1, "gb1": gb1_, "gw2": gw2, "gb2": gb2_,
            "w1s": w1s, "b1r": b1r, "w2r": w2r, "b2r": b2r,
        })
    return in_maps


_NC_CACHE = {}


def kernel(**inputs) -> np.ndarray:
    key = "full"
    if key not in _NC_CACHE:
        _NC_CACHE[key] = build_nc()
    nc = _NC_CACHE[key]
    in_maps = _prep_inputs(**inputs)
    res = run_bass_kernel_spmd(nc, in_maps, core_ids=list(range(NCORES)))
    out = np.concatenate([res.results[c]["out"] for c in range(NCORES)], axis=0)
    return out.astype(np.float32)



# revision 3
# speedup vs baseline: 1.0370x; 1.0370x over previous
"""Trainium2 Bass kernel for CNN+GCN+MLP (nn_CNNGCN_18236431139458).

Strategy (8 NeuronCores, one chip), v2:
  - Data-parallel over batch (4 samples/core) for conv + both GCN layers.
  - The scatter-aggregate is a dense matmul against A^T (built host-side
    from edge_index), resident in SBUF as fp8 (x16 scale), shared by
    BOTH layers, run in DoubleRow fp8 (2x PE rate):
      layer 1: original order (linear then aggregate), as before.
      layer 2: swapped order  (aggregate h1 first, then linear) --
        h1 >= 0 post-relu, so the aggregation is a positive-weighted sum
        and fp8 quantization noise averages away (~1e-2 rel-l2 vs the
        2e-2 gate, validated host-side).
  - agg2 -> lin2 -> AllToAll pipelined per sample so the reshard hides
    under the next sample's compute.
  - MLP: W1 row-sharded over nodes across cores (fully SBUF-resident,
    loaded once); AllToAll delivers node-sharded h2; each core computes
    a partial [32, 100]; ReduceScatter sums and scatters; tiny local
    MLP tail.
  - DMA split: big streams (A^T, W1 shard) on GpSimd/SWDGE; input +
    staging + latency-critical transfers on Sync/HWDGE.

Layouts (per core):
  xT   [128 ic, 4 s, 2050]       feature-major input slices
  h0T  [128 oc, 4 s, 2048 n]     conv output (feature partitions)
  hw1p [128 n, 16 nch, 4sx128g]  fp8(64 * h0 @ gW1)   (node partitions)
  at8  [128 src, 16 chunk, 2048] fp8(16 * A^T)        (resident)
  h1p  [128 n, 16 nch, 4sx128g]  fp8(64 * relu(agg1 + b1))
  a2   [128 g, 4 s, 2048 dst]    bf16(A^T @ h1)       (feature partitions)
  h2   [128 n, 16 nt, 4 s, 128]  bf16 relu(a2 @ gW2 + b2) (node partitions)
  h2a  [128 n, 2 nt, 32 b, 128]  A2A landing (this core's 256-node shard)
  w1x  [128 n, 2 nt, 128 f, 100] W1 row-shard, reordered (resident)
"""

import numpy as np
import ml_dtypes

import concourse.bass as bass
import concourse.mybir as mybir
import concourse.tile as tile
from concourse import bacc
from concourse.bass_utils import run_bass_kernel_spmd

BF16 = mybir.dt.bfloat16
FP8 = mybir.dt.float8e4
F32 = mybir.dt.float32
NP_BF16 = ml_dtypes.bfloat16
NP_FP8 = mybir.dt.np(FP8)

B, H, E = 32, 2050, 128
N = 2048
C = 128
G1 = G2 = 128
MLPD = 100
KS = 3
NCORES = 8
BL = B // NCORES          # 4 samples per core
NSH = N // NCORES         # 256 nodes per core (W1 row shard)
RG = [list(range(NCORES))]

S_HW = 64.0               # fp8 scale for hw1 / h1
S_AT = 16.0               # fp8 scale for A^T
S_AGG = S_HW * S_AT       # psum scale after fp8 x fp8 aggregation

Relu = mybir.ActivationFunctionType.Relu
DoubleRow = mybir.MatmulPerfMode.DoubleRow


def build_nc(n_warm=32, collectives=True):
    nc = bacc.Bacc("TRN2", target_bir_lowering=False, debug=False,
                   num_devices=NCORES)

    d_xT = nc.dram_tensor("xT", [BL, 128, H], BF16, kind="ExternalInput").ap()
    d_at8 = nc.dram_tensor("at8", [16, 128, N], FP8, kind="ExternalInput").ap()
    d_wc = nc.dram_tensor("wc", [KS, 128, 128], BF16, kind="ExternalInput").ap()
    d_cb = nc.dram_tensor("cb", [128, 1], F32, kind="ExternalInput").ap()
    d_gw1 = nc.dram_tensor("gw1", [128, 128], BF16, kind="ExternalInput").ap()
    d_gb1r = nc.dram_tensor("gb1r", [128, 512], F32, kind="ExternalInput").ap()
    d_gw2 = nc.dram_tensor("gw2", [128, 128], BF16, kind="ExternalInput").ap()
    d_gb2r = nc.dram_tensor("gb2r", [128, 4, 128], F32, kind="ExternalInput").ap()
    d_w1x = nc.dram_tensor("w1x", [128, 2, 128, MLPD], BF16, kind="ExternalInput").ap()
    d_b1r = nc.dram_tensor("b1r", [BL, MLPD], F32, kind="ExternalInput").ap()
    d_w2r = nc.dram_tensor("w2r", [BL, MLPD], F32, kind="ExternalInput").ap()
    d_b2r = nc.dram_tensor("b2r", [BL, 1], F32, kind="ExternalInput").ap()
    d_out = nc.dram_tensor("out", [BL, 1], F32, kind="ExternalOutput").ap()

    with tile.TileContext(nc) as tc:
        with (
            tc.tile_pool(name="const", bufs=1) as const,
            tc.tile_pool(name="big", bufs=1) as big,
            tc.tile_pool(name="tmp", bufs=3) as tmp,
            tc.tile_pool(name="small", bufs=1) as small,
            tc.tile_pool(name="psum", bufs=7, space="PSUM") as psum,
            tc.tile_pool(name="psum2", bufs=1, space="PSUM") as psum2,
            tc.tile_pool(name="dram", bufs=1, space="DRAM") as dram,
        ):
            # ---- SBUF loads ----
            # sync/HWDGE: latency-critical small inputs + x
            xT_sb = big.tile([128, BL, H], BF16, tag="xt_a2", bufs=1, name="xT_sb")
            for s in range(BL):
                nc.sync.dma_start(xT_sb[:, s, :], d_xT[s])
            wc_sb = const.tile([128, KS, 128], BF16, tag="wc", name="wc_sb")
            nc.sync.dma_start(wc_sb[:], d_wc.rearrange("k p o -> p k o"))
            cb_sb = const.tile([128, 1], F32, tag="cb", name="cb_sb")
            nc.sync.dma_start(cb_sb[:], d_cb[:])
            gw1_sb = const.tile([128, 128], BF16, tag="gw1", name="gw1_sb")
            nc.sync.dma_start(gw1_sb[:], d_gw1[:])
            gw2_sb = const.tile([128, 128], BF16, tag="gw2", name="gw2_sb")
            nc.sync.dma_start(gw2_sb[:], d_gw2[:])
            gb1r_sb = const.tile([128, 512], F32, tag="gb1r", name="gb1r_sb")
            nc.sync.dma_start(gb1r_sb[:], d_gb1r[:])
            gb2r_sb = const.tile([128, 4, 128], F32, tag="gb2r", name="gb2r_sb")
            nc.sync.dma_start(gb2r_sb[:], d_gb2r[:])
            b1r_sb = small.tile([BL, MLPD], F32, tag="b1r", name="b1r_sb")
            nc.sync.dma_start(b1r_sb[:], d_b1r[:])
            w2r_sb = small.tile([BL, MLPD], F32, tag="w2r", name="w2r_sb")
            nc.sync.dma_start(w2r_sb[:], d_w2r[:])
            b2r_sb = small.tile([BL, 1], F32, tag="b2r", name="b2r_sb")
            nc.sync.dma_start(b2r_sb[:], d_b2r[:])

            # gpsimd/SWDGE: big resident streams (A^T fp8 first, then W1)
            at8_sb = const.tile([128, 16, N], FP8, tag="at8", name="at8_sb")
            for q in range(4):
                nc.gpsimd.dma_start(
                    at8_sb[:, 4 * q : 4 * q + 4, :],
                    d_at8[4 * q : 4 * q + 4].rearrange("c p d -> p c d"),
                )
            w1x_sb = const.tile([128, 2, 128, MLPD], BF16, tag="w1x", name="w1x_sb")
            for q in range(2):
                nc.gpsimd.dma_start(w1x_sb[:, q, :, :], d_w1x[:, q, :, :])

            # ---- conv: h0T[oc, n] = relu(sum_k WcT_k.T @ xT[:, n+k] + cb),
            #      sample-outer so conv(s0) starts as soon as x(s0) lands ----
            h0T = big.tile([128, BL, N], BF16, tag="h0_h2a", bufs=1, name="h0T")
            for s in range(BL):
                for nt in range(4):
                    ps = psum.tile([128, 512], F32, tag="ps", name="ps_conv")
                    for k in range(KS):
                        nc.tensor.matmul(
                            ps[:],
                            lhsT=wc_sb[:, k, :],
                            rhs=xT_sb[:, s, nt * 512 + k : nt * 512 + k + 512],
                            start=(k == 0),
                            stop=(k == KS - 1),
                        )
                    nc.scalar.activation(h0T[:, s, nt * 512 : (nt + 1) * 512],
                                         ps[:], Relu, bias=cb_sb[:])

            # ---- lin1: hw1p[n, (s,g)] = fp8(64 * h0 @ gW1) ----
            hw1p = big.tile([128, 16, 512], FP8, tag="hw1p", bufs=1, name="hw1p")
            for nch in range(16):
                ps = psum.tile([128, 512], F32, tag="ps", name="ps_lin1")
                for s in range(BL):
                    nc.tensor.matmul(
                        ps[:, s * 128 : (s + 1) * 128],
                        lhsT=h0T[:, s, nch * 128 : (nch + 1) * 128],
                        rhs=gw1_sb[:],
                        start=True,
                        stop=True,
                    )
                nc.vector.tensor_scalar_mul(hw1p[:, nch, :], ps[:], S_HW)

            # ---- agg1 (orientation-swapped, fp8 DoubleRow):
            #      out[dst, (s,g)] = sum_src at8[src, dst] * hw1p[src, (s,g)]
            #      h1p = fp8(relu((psum + 1024*b1row) / 16)) = fp8(64 * h1) ----
            h1p = big.tile([128, 16, 512], FP8, tag="h1p", bufs=1, name="h1p")
            for dt in range(16):
                ps = psum.tile([128, 512], F32, tag="ps", name="ps_agg1")
                for sc2 in range(8):
                    nc.tensor.matmul(
                        ps[:],
                        lhsT=at8_sb[:, 2 * sc2 : 2 * sc2 + 2, dt * 128 : (dt + 1) * 128],
                        rhs=hw1p[:, 2 * sc2 : 2 * sc2 + 2, :],
                        start=(sc2 == 0),
                        stop=(sc2 == 7),
                        perf_mode=DoubleRow,
                    )
                t1 = tmp.tile([128, 512], F32, tag="t1", bufs=3, name="t1")
                nc.vector.tensor_add(t1[:], ps[:], gb1r_sb[:])
                nc.scalar.activation(h1p[:, dt, :], t1[:], Relu, scale=1.0 / S_AT)

            # ---- per-sample: agg2 (swapped order, fp8 DR) -> lin2 -> A2A ----
            a2 = big.tile([128, BL, N], BF16, tag="xt_a2", bufs=1, name="a2")
            h2 = big.tile([128, 16, BL, 128], BF16, tag="h2", bufs=1, name="h2")
            h2a = big.tile([128, 2, B, 128], BF16, tag="h0_h2a", bufs=1, name="h2a")
            for s in range(BL):
                # agg2: a2[g, dst] = sum_src at8[src, dst] * h1p[src, (s,g)]
                for dt in range(4):
                    ps = psum.tile([128, 512], F32, tag="ps", name="ps_agg2")
                    for sc2 in range(8):
                        nc.tensor.matmul(
                            ps[:],
                            lhsT=h1p[:, 2 * sc2 : 2 * sc2 + 2, s * 128 : (s + 1) * 128],
                            rhs=at8_sb[:, 2 * sc2 : 2 * sc2 + 2, dt * 512 : (dt + 1) * 512],
                            start=(sc2 == 0),
                            stop=(sc2 == 7),
                            perf_mode=DoubleRow,
                        )
                    nc.vector.tensor_scalar_mul(a2[:, s, dt * 512 : (dt + 1) * 512],
                                                ps[:], 1.0 / S_AGG)
                # lin2: h2[dst, g2] = relu(a2[:, dst].T @ gW2 + b2row)
                for ntg in range(4):
                    ps2 = psum.tile([128, 4, 128], F32, tag="ps", name="ps_lin2")
                    for ntl in range(4):
                        nt = ntg * 4 + ntl
                        nc.tensor.matmul(
                            ps2[:, ntl, :],
                            lhsT=a2[:, s, nt * 128 : (nt + 1) * 128],
                            rhs=gw2_sb[:],
                            start=True,
                            stop=True,
                        )
                    t2 = tmp.tile([128, 4, 128], F32, tag="t2", bufs=3, name="t2")
                    nc.vector.tensor_add(t2[:], ps2[:], gb2r_sb[:])
                    nc.scalar.activation(h2[:, 4 * ntg : 4 * ntg + 4, s, :], t2[:], Relu)
                # A2A(s): send node-shard j of sample s to core j
                a2a_in = dram.tile([NCORES, 128, 2, 128], BF16, tag=f"a2a_in{s}",
                                   name=f"a2a_in{s}")
                a2a_out = dram.tile([NCORES, 128, 2, 128], BF16, tag=f"a2a_out{s}",
                                    name=f"a2a_out{s}")
                for j in range(NCORES):
                    nc.sync.dma_start(a2a_in[j], h2[:, 2 * j : 2 * j + 2, s, :])
                if collectives:
                    nc.gpsimd.collective_compute(
                        "AllToAll", mybir.AluOpType.bypass, replica_groups=RG,
                        ins=[a2a_in.opt()], outs=[a2a_out.opt()],
                    )
                    for i in range(NCORES):
                        nc.sync.dma_start(h2a[:, :, i * BL + s, :], a2a_out[i])
                else:
                    for i in range(NCORES):
                        nc.sync.dma_start(h2a[:, :, i * BL + s, :], a2a_in[i])

            # ---- PE warmers across the A2A wait (keep HAM at full clock) ----
            if n_warm > 0:
                warm_ps = psum.tile([128, 512], F32, tag="ps", name="warm_ps")
                for w in range(n_warm):
                    nc.tensor.matmul(
                        warm_ps[:],
                        lhsT=gw2_sb[:],
                        rhs=a2[:, BL - 1, 0:512],
                        start=(w == 0),
                        stop=(w == n_warm - 1),
                    )
                warm_sb = small.tile([128, 1], F32, tag="warm_sb", name="warm_sb")
                nc.vector.tensor_copy(warm_sb[:], warm_ps[:, 0:1])
                warm_dr = dram.tile([128, 1], F32, tag="warm_dr", name="warm_dr")
                nc.sync.dma_start(warm_dr[:], warm_sb[:])

            # ---- z[b, c] = sum_{nt,f} h2a[:, nt, :, f].T @ w1x[:, nt, f, :]
            #      4 col-groups (tile_position) keep the PE array full ----
            ps_z = psum2.tile([128, MLPD], F32, tag="psz", name="ps_z")
            for nt in range(2):
                for f in range(128):
                    k = nt * 128 + f
                    j = k % 4
                    nc.tensor.matmul(
                        ps_z[32 * j : 32 * (j + 1), :],
                        lhsT=h2a[:, nt, :, f],
                        rhs=w1x_sb[:, nt, f, :],
                        start=(k < 4),
                        stop=(k >= 252),
                        tile_position=(0, 32 * j),
                    )

            z_sb = small.tile([32, MLPD], F32, tag="z_sb", name="z_sb")
            nc.vector.tensor_copy(z_sb[:], ps_z[0:32, :])
            for j in range(1, 4):
                nc.vector.tensor_add(z_sb[:], z_sb[:], ps_z[32 * j : 32 * (j + 1), :])
            rs_in = dram.tile([32, MLPD], F32, tag="rs_in", name="rs_in")
            rs_out = dram.tile([BL, MLPD], F32, tag="rs_out", name="rs_out")
            nc.sync.dma_start(rs_in[:], z_sb[:])
            zloc = small.tile([BL, MLPD], F32, tag="zloc", name="zloc")
            if collectives:
                nc.gpsimd.collective_compute(
                    "ReduceScatter", mybir.AluOpType.add, replica_groups=RG,
                    ins=[rs_in.opt()], outs=[rs_out.opt()],
                )
                nc.sync.dma_start(zloc[:], rs_out[:])
            else:
                nc.sync.dma_start(zloc[:], rs_in[0:BL, :])
            hm = small.tile([BL, MLPD], F32, tag="hm", name="hm")
            nc.vector.tensor_add(hm[:], zloc[:], b1r_sb[:])
            nc.vector.tensor_scalar_max(hm[:], hm[:], 0.0)
            nc.vector.tensor_mul(hm[:], hm[:], w2r_sb[:])
            osb = small.tile([BL, 1], F32, tag="osb", name="osb")
            nc.vector.reduce_sum(osb[:], hm[:], axis=mybir.AxisListType.X)
            nc.vector.tensor_add(osb[:], osb[:], b2r_sb[:])
            nc.sync.dma_start(d_out[:], osb[:])

    nc.compile()
    return nc


def _prep_inputs(x, edge_index, conv_w, conv_b, gW1, gb1, gW2, gb2, W1, b1, W2, b2):
    """Host-side sharding / layout prep -> per-core input maps."""
    # gcn_norm (add_self_loops=True), duplicated edges accumulate
    src = np.concatenate([np.asarray(edge_index[0]), np.arange(N, dtype=np.int64)])
    dst = np.concatenate([np.asarray(edge_index[1]), np.arange(N, dtype=np.int64)])
    deg = np.bincount(dst, minlength=N).astype(np.float32)
    dinv = (1.0 / np.sqrt(np.maximum(deg, 1.0))).astype(np.float32)
    normv = dinv[src] * dinv[dst]
    AT = np.zeros((N, N), np.float32)
    np.add.at(AT, (src, dst), normv)
    at8 = np.ascontiguousarray((AT * S_AT).reshape(16, 128, N)).astype(NP_FP8)

    wc = np.ascontiguousarray(
        np.asarray(conv_w)[:, 0, :, :].transpose(1, 2, 0)  # [KS, ic, oc]
    ).astype(NP_BF16)
    cb = np.asarray(conv_b, np.float32).reshape(128, 1)
    gw1 = np.asarray(gW1).astype(NP_BF16)
    gw2 = np.asarray(gW2).astype(NP_BF16)
    gb1r = np.ascontiguousarray(np.broadcast_to(
        np.tile(np.asarray(gb1, np.float32) * (S_AGG / 1.0), BL), (128, 512)))
    gb2r = np.ascontiguousarray(np.broadcast_to(
        np.asarray(gb2, np.float32)[None, None, :], (128, 4, 128)))
    b1r = np.ascontiguousarray(np.broadcast_to(np.asarray(b1, np.float32), (BL, MLPD)))
    w2r = np.ascontiguousarray(np.broadcast_to(np.asarray(W2, np.float32)[:, 0], (BL, MLPD)))
    b2r = np.full((BL, 1), np.asarray(b2, np.float32)[0], np.float32)

    W1r = np.asarray(W1, np.float32).reshape(N, G2, MLPD)
    x_np = np.asarray(x, np.float32)

    in_maps = []
    for c in range(NCORES):
        xT = np.ascontiguousarray(
            x_np[c * BL : (c + 1) * BL].transpose(0, 2, 1)
        ).astype(NP_BF16)
        w1x = np.ascontiguousarray(
            W1r[c * NSH : (c + 1) * NSH].reshape(2, 128, G2, MLPD).transpose(1, 0, 2, 3)
        ).astype(NP_BF16)
        in_maps.append({
            "xT": xT, "at8": at8, "wc": wc, "cb": cb,
            "gw1": gw1, "gb1r": gb1r, "gw2": gw2, "gb2r": gb2r,
            "w1x": w1x, "b1r": b1r, "w2r": w2r, "b2r": b2r,
        })
    return in_maps


_NC_CACHE = {}


def kernel(**inputs) -> np.ndarray:
    key = "full"
    if key not in _NC_CACHE:
        _NC_CACHE[key] = build_nc()
    nc = _NC_CACHE[key]
    in_maps = _prep_inputs(**inputs)
    res = run_bass_kernel_spmd(nc, in_maps, core_ids=list(range(NCORES)))
    out = np.concatenate([res.results[c]["out"] for c in range(NCORES)], axis=0)
    return out.astype(np.float32)


# revision 8
# speedup vs baseline: 1.0989x; 1.0597x over previous
"""Trainium2 Bass kernel for CNN+GCN+MLP (nn_CNNGCN_18236431139458).

Strategy (8 NeuronCores, one chip), v2:
  - Data-parallel over batch (4 samples/core) for conv + both GCN layers.
  - The scatter-aggregate is a dense matmul against A^T (built host-side
    from edge_index), resident in SBUF as fp8 (x16 scale), shared by
    BOTH layers, run in DoubleRow fp8 (2x PE rate):
      layer 1: original order (linear then aggregate), as before.
      layer 2: swapped order  (aggregate h1 first, then linear) --
        h1 >= 0 post-relu, so the aggregation is a positive-weighted sum
        and fp8 quantization noise averages away (~1e-2 rel-l2 vs the
        2e-2 gate, validated host-side).
  - agg2 -> lin2 -> AllToAll pipelined per sample so the reshard hides
    under the next sample's compute.
  - MLP: W1 row-sharded over nodes across cores (fully SBUF-resident,
    loaded once); AllToAll delivers node-sharded h2; each core computes
    a partial [32, 100]; ReduceScatter sums and scatters; tiny local
    MLP tail.
  - DMA split: big streams (A^T, W1 shard) on GpSimd/SWDGE; input +
    staging + latency-critical transfers on Sync/HWDGE.

Layouts (per core):
  xT   [128 ic, 4 s, 2050]       feature-major input slices
  h0T  [128 oc, 4 s, 2048 n]     conv output (feature partitions)
  hw1p [128 n, 16 nch, 4sx128g]  fp8(64 * h0 @ gW1)   (node partitions)
  at8  [128 src, 16 chunk, 2048] fp8(16 * A^T)        (resident)
  h1p  [128 n, 16 nch, 4sx128g]  fp8(64 * relu(agg1 + b1))
  a2   [128 g, 4 s, 2048 dst]    bf16(A^T @ h1)       (feature partitions)
  h2   [128 n, 16 nt, 4 s, 128]  bf16 relu(a2 @ gW2 + b2) (node partitions)
  h2a  [128 n, 2 nt, 32 b, 128]  A2A landing (this core's 256-node shard)
  w1x  [128 n, 2 nt, 128 f, 100] W1 row-shard, reordered (resident)
"""

import numpy as np
import ml_dtypes

import concourse.bass as bass
import concourse.mybir as mybir
import concourse.tile as tile
from concourse.tile import add_dep_helper
from concourse import bacc
from concourse.bass_utils import run_bass_kernel_spmd

BF16 = mybir.dt.bfloat16
FP8 = mybir.dt.float8e4
F32 = mybir.dt.float32
NP_BF16 = ml_dtypes.bfloat16
NP_FP8 = mybir.dt.np(FP8)

B, H, E = 32, 2050, 128
N = 2048
C = 128
G1 = G2 = 128
MLPD = 100
KS = 3
NCORES = 8
BL = B // NCORES          # 4 samples per core
NSH = N // NCORES         # 256 nodes per core (W1 row shard)
RG = [list(range(NCORES))]

S_HW = 64.0               # fp8 scale for hw1 / h1
S_AT = 16.0               # fp8 scale for A^T
S_AGG = S_HW * S_AT       # psum scale after fp8 x fp8 aggregation

Relu = mybir.ActivationFunctionType.Relu
DoubleRow = mybir.MatmulPerfMode.DoubleRow


def build_nc(n_warm=48, collectives=True):
    nc = bacc.Bacc("TRN2", target_bir_lowering=False, debug=False,
                   num_devices=NCORES)

    d_xT = nc.dram_tensor("xT", [BL, 128, H], BF16, kind="ExternalInput").ap()
    d_at8 = nc.dram_tensor("at8", [16, 128, N], FP8, kind="ExternalInput").ap()
    d_wc = nc.dram_tensor("wc", [128, KS, 128], BF16, kind="ExternalInput").ap()
    d_cb = nc.dram_tensor("cb", [128, 1], F32, kind="ExternalInput").ap()
    d_gw1 = nc.dram_tensor("gw1", [128, 128], BF16, kind="ExternalInput").ap()
    d_gb1r = nc.dram_tensor("gb1r", [128, 512], F32, kind="ExternalInput").ap()
    d_gw2 = nc.dram_tensor("gw2", [128, 128], BF16, kind="ExternalInput").ap()
    d_gb2r = nc.dram_tensor("gb2r", [128, 2, 256], F32, kind="ExternalInput").ap()
    d_w1x = nc.dram_tensor("w1x", [128, 2, 128, MLPD], BF16, kind="ExternalInput").ap()
    d_b1r = nc.dram_tensor("b1r", [BL, MLPD], F32, kind="ExternalInput").ap()
    d_w2r = nc.dram_tensor("w2r", [BL, MLPD], F32, kind="ExternalInput").ap()
    d_b2r = nc.dram_tensor("b2r", [BL, 1], F32, kind="ExternalInput").ap()
    d_out = nc.dram_tensor("out", [BL, 1], F32, kind="ExternalOutput").ap()

    with tile.TileContext(nc) as tc:
        with (
            tc.tile_pool(name="const", bufs=1) as const,
            tc.tile_pool(name="big", bufs=1) as big,
            tc.tile_pool(name="tmp", bufs=3) as tmp,
            tc.tile_pool(name="small", bufs=1) as small,
            tc.tile_pool(name="psum", bufs=7, space="PSUM") as psum,
            tc.tile_pool(name="psum2", bufs=1, space="PSUM") as psum2,
            tc.tile_pool(name="dram", bufs=1, space="DRAM") as dram,
        ):
            # ---- SBUF loads ----
            # sync/HWDGE: conv weights first (conv stationary unblocks at
            # ~1us), then x, then later-needed smalls
            wc_sb = const.tile([128, KS, 128], BF16, tag="wc", name="wc_sb")
            nc.sync.dma_start(wc_sb[:], d_wc[:])
            cb_sb = const.tile([128, 1], F32, tag="cb", name="cb_sb")
            nc.sync.dma_start(cb_sb[:], d_cb[:])
            gw1_sb = const.tile([128, 128], BF16, tag="gw1", name="gw1_sb")
            nc.sync.dma_start(gw1_sb[:], d_gw1[:])
            xT_sb = big.tile([128, BL, H], BF16, tag="xt_a2", bufs=1, name="xT_sb")
            for s in range(BL):
                nc.sync.dma_start(xT_sb[:, s, :], d_xT[s])
            gw2_sb = const.tile([128, 128], BF16, tag="gw2", name="gw2_sb")
            nc.sync.dma_start(gw2_sb[:], d_gw2[:])
            gb1r_sb = const.tile([128, 512], F32, tag="gb1r", name="gb1r_sb")
            nc.sync.dma_start(gb1r_sb[:], d_gb1r[:])
            gb2r_sb = const.tile([128, 2, 256], F32, tag="gb2r", name="gb2r_sb")
            nc.sync.dma_start(gb2r_sb[:], d_gb2r[:])
            b1r_sb = small.tile([BL, MLPD], F32, tag="b1r", name="b1r_sb")
            nc.sync.dma_start(b1r_sb[:], d_b1r[:])
            w2r_sb = small.tile([BL, MLPD], F32, tag="w2r", name="w2r_sb")
            nc.sync.dma_start(w2r_sb[:], d_w2r[:])
            b2r_sb = small.tile([BL, 1], F32, tag="b2r", name="b2r_sb")
            nc.sync.dma_start(b2r_sb[:], d_b2r[:])

            # gpsimd/SWDGE: big resident streams (A^T fp8 first, then W1)
            at8_sb = const.tile([128, 16, N], FP8, tag="at8", name="at8_sb")
            for q in range(4):
                nc.gpsimd.dma_start(
                    at8_sb[:, 4 * q : 4 * q + 4, :],
                    d_at8[4 * q : 4 * q + 4].rearrange("c p d -> p c d"),
                )
            w1x_sb = const.tile([128, 2, 128, MLPD], BF16, tag="w1x", name="w1x_sb")
            w1x_dmas = [nc.gpsimd.dma_start(w1x_sb[:, q, :, :], d_w1x[:, q, :, :])
                        for q in range(2)]

            # ---- conv: h0T[oc, n] = relu(sum_k WcT_k.T @ xT[:, n+k] + cb),
            #      sample-outer so conv(s0) starts as soon as x(s0) lands ----
            h0T = big.tile([128, BL, N], BF16, tag="h0_h2a", bufs=1, name="h0T")
            for s in range(BL):
                for nt in range(4):
                    ps = psum.tile([128, 512], F32, tag="ps", name="ps_conv")
                    for k in range(KS):
                        nc.tensor.matmul(
                            ps[:],
                            lhsT=wc_sb[:, k, :],
                            rhs=xT_sb[:, s, nt * 512 + k : nt * 512 + k + 512],
                            start=(k == 0),
                            stop=(k == KS - 1),
                        )
                    act = nc.scalar.activation(h0T[:, s, nt * 512 : (nt + 1) * 512],
                                               ps[:], Relu, bias=cb_sb[:])
                    if s == 0 and nt == 0:
                        for dm in w1x_dmas:
                            add_dep_helper(dm.ins, act.ins,
                                           reason="delay W1 load past startup crunch")

            # ---- lin1: hw1p[n, (s,g)] = fp8(64 * h0 @ gW1) ----
            hw1p = big.tile([128, 16, 512], FP8, tag="hw1p", bufs=1, name="hw1p")
            for nch in range(16):
                ps = psum.tile([128, 512], F32, tag="ps", name="ps_lin1")
                for s in range(BL):
                    nc.tensor.matmul(
                        ps[:, s * 128 : (s + 1) * 128],
                        lhsT=h0T[:, s, nch * 128 : (nch + 1) * 128],
                        rhs=gw1_sb[:],
                        start=True,
                        stop=True,
                    )
                nc.vector.tensor_scalar_mul(hw1p[:, nch, :], ps[:], S_HW)

            # ---- agg1 (orientation-swapped, fp8 DoubleRow):
            #      out[dst, (s,g)] = sum_src at8[src, dst] * hw1p[src, (s,g)]
            #      h1p = fp8(relu((psum + 1024*b1row) / 16)) = fp8(64 * h1) ----
            h1p = big.tile([128, 16, 512], FP8, tag="h1p", bufs=1, name="h1p")
            for dt in range(16):
                ps = psum.tile([128, 512], F32, tag="ps", name="ps_agg1")
                for sc2 in range(8):
                    nc.tensor.matmul(
                        ps[:],
                        lhsT=at8_sb[:, 2 * sc2 : 2 * sc2 + 2, dt * 128 : (dt + 1) * 128],
                        rhs=hw1p[:, 2 * sc2 : 2 * sc2 + 2, :],
                        start=(sc2 == 0),
                        stop=(sc2 == 7),
                        perf_mode=DoubleRow,
                    )
                t1 = tmp.tile([128, 512], F32, tag="t1", bufs=3, name="t1")
                nc.vector.tensor_add(t1[:], ps[:], gb1r_sb[:])
                nc.scalar.activation(h1p[:, dt, :], t1[:], Relu, scale=1.0 / S_AT)

            # ---- per-sample: agg2 (swapped order, fp8 DR) -> lin2 -> A2A ----
            a2 = big.tile([128, BL, N], BF16, tag="xt_a2", bufs=1, name="a2")
            h2 = big.tile([128, BL, 8, 256], BF16, tag="h2", bufs=1, name="h2")
            # h2a: A2A landing, batch order permuted to (s_local, src_core)
            h2a = big.tile([128, BL, NCORES, 256], BF16, tag="h0_h2a", bufs=1,
                           name="h2a")
            a2a_bufs = []
            for s in range(BL):
                # agg2: a2[g, dst] = sum_src at8[src, dst] * h1p[src, (s,g)]
                for dt in range(4):
                    ps = psum.tile([128, 512], F32, tag="ps", name="ps_agg2")
                    for sc2 in range(8):
                        nc.tensor.matmul(
                            ps[:],
                            lhsT=h1p[:, 2 * sc2 : 2 * sc2 + 2, s * 128 : (s + 1) * 128],
                            rhs=at8_sb[:, 2 * sc2 : 2 * sc2 + 2, dt * 512 : (dt + 1) * 512],
                            start=(sc2 == 0),
                            stop=(sc2 == 7),
                            perf_mode=DoubleRow,
                        )
                    nc.vector.tensor_scalar_mul(a2[:, s, dt * 512 : (dt + 1) * 512],
                                                ps[:], 1.0 / S_AGG)
                # lin2: h2[dst, g2] = relu(a2[:, dst].T @ gW2 + b2row)
                for ntg in range(4):
                    ps2 = psum.tile([128, 2, 256], F32, tag="ps", name="ps_lin2")
                    for ntl in range(4):
                        nt = ntg * 4 + ntl
                        nc.tensor.matmul(
                            ps2[:, ntl // 2, (ntl % 2) * 128 : (ntl % 2) * 128 + 128],
                            lhsT=a2[:, s, nt * 128 : (nt + 1) * 128],
                            rhs=gw2_sb[:],
                            start=True,
                            stop=True,
                        )
                    t2 = tmp.tile([128, 2, 256], F32, tag="t2", bufs=3, name="t2")
                    nc.vector.tensor_add(t2[:], ps2[:], gb2r_sb[:])
                    nc.scalar.activation(h2[:, s, 2 * ntg : 2 * ntg + 2, :],
                                         t2[:], Relu)
                # A2A(s): one coarse staging DMA, trigger; returns drained
                # after the loop so the sync queue never blocks on the wire
                a2a_in = dram.tile([NCORES, 128, 256], BF16, tag=f"a2a_in{s}",
                                   name=f"a2a_in{s}")
                a2a_out = dram.tile([NCORES, 128, 256], BF16, tag=f"a2a_out{s}",
                                    name=f"a2a_out{s}")
                nc.sync.dma_start(a2a_in.rearrange("j p m -> p j m"),
                                  h2[:, s, :, :])
                if collectives:
                    nc.gpsimd.collective_compute(
                        "AllToAll", mybir.AluOpType.bypass, replica_groups=RG,
                        ins=[a2a_in.opt()], outs=[a2a_out.opt()],
                    )
                a2a_bufs.append((a2a_in, a2a_out))

            # drain the A2A results (one DMA per sample)
            for s in range(BL):
                a2a_in, a2a_out = a2a_bufs[s]
                srcbuf = a2a_out if collectives else a2a_in
                nc.sync.dma_start(h2a[:, s, :, :],
                                  srcbuf.rearrange("i p m -> p i m"))

            # ---- PE warmers across the A2A wait (keep HAM at full clock) ----
            if n_warm > 0:
                warm_ps = psum.tile([128, 512], F32, tag="ps", name="warm_ps")
                for w in range(n_warm):
                    nc.tensor.matmul(
                        warm_ps[:],
                        lhsT=gw2_sb[:],
                        rhs=a2[:, BL - 1, 0:512],
                        start=(w == 0),
                        stop=(w == n_warm - 1),
                    )
                warm_sb = small.tile([128, 1], F32, tag="warm_sb", name="warm_sb")
                nc.vector.tensor_copy(warm_sb[:], warm_ps[:, 0:1])
                warm_dr = dram.tile([128, 1], F32, tag="warm_dr", name="warm_dr")
                nc.sync.dma_start(warm_dr[:], warm_sb[:])

            # ---- z[b, c] = sum_{nt,f} h2a[:, nt, :, f].T @ w1x[:, nt, f, :]
            #      4 col-groups (tile_position) keep the PE array full ----
            ps_z = psum2.tile([128, MLPD], F32, tag="psz", name="ps_z")
            for nt in range(2):
                for f in range(128):
                    k = nt * 128 + f
                    j = k % 4
                    nc.tensor.matmul(
                        ps_z[32 * j : 32 * (j + 1), :],
                        lhsT=h2a[:, :, :, nt * 128 + f],
                        rhs=w1x_sb[:, nt, f, :],
                        start=(k < 4),
                        stop=(k >= 252),
                        tile_position=(0, 32 * j),
                    )

            z_sb = small.tile([32, MLPD], F32, tag="z_sb", name="z_sb")
            nc.vector.tensor_copy(z_sb[:], ps_z[0:32, :])
            for j in range(1, 4):
                nc.vector.tensor_add(z_sb[:], z_sb[:], ps_z[32 * j : 32 * (j + 1), :])
            rs_in = dram.tile([32, MLPD], F32, tag="rs_in", name="rs_in")
            rs_out = dram.tile([BL, MLPD], F32, tag="rs_out", name="rs_out")
            nc.sync.dma_start(rs_in[:], z_sb[:])
            zloc = small.tile([BL, MLPD], F32, tag="zloc", name="zloc")
            if collectives:
                nc.gpsimd.collective_compute(
                    "ReduceScatter", mybir.AluOpType.add, replica_groups=RG,
                    ins=[rs_in.opt()], outs=[rs_out.opt()],
                )
                nc.sync.dma_start(zloc[:], rs_out[:])
            else:
                nc.sync.dma_start(zloc[:], rs_in[0:BL, :])
            hm = small.tile([BL, MLPD], F32, tag="hm", name="hm")
            nc.vector.tensor_add(hm[:], zloc[:], b1r_sb[:])
            nc.vector.tensor_scalar_max(hm[:], hm[:], 0.0)
            nc.vector.tensor_mul(hm[:], hm[:], w2r_sb[:])
            osb = small.tile([BL, 1], F32, tag="osb", name="osb")
            nc.vector.reduce_sum(osb[:], hm[:], axis=mybir.AxisListType.X)
            nc.vector.tensor_add(osb[:], osb[:], b2r_sb[:])
            nc.sync.dma_start(d_out[:], osb[:])

    nc.compile()
    return nc


def _prep_inputs(x, edge_index, conv_w, conv_b, gW1, gb1, gW2, gb2, W1, b1, W2, b2):
    """Host-side sharding / layout prep -> per-core input maps."""
    # gcn_norm (add_self_loops=True), duplicated edges accumulate
    src = np.concatenate([np.asarray(edge_index[0]), np.arange(N, dtype=np.int64)])
    dst = np.concatenate([np.asarray(edge_index[1]), np.arange(N, dtype=np.int64)])
    deg = np.bincount(dst, minlength=N).astype(np.float32)
    dinv = (1.0 / np.sqrt(np.maximum(deg, 1.0))).astype(np.float32)
    normv = dinv[src] * dinv[dst]
    AT = np.zeros((N, N), np.float32)
    np.add.at(AT, (src, dst), normv)
    at8 = np.ascontiguousarray((AT * S_AT).reshape(16, 128, N)).astype(NP_FP8)

    wc = np.ascontiguousarray(
        np.asarray(conv_w)[:, 0, :, :].transpose(2, 1, 0)  # [ic, KS, oc]
    ).astype(NP_BF16)
    cb = np.asarray(conv_b, np.float32).reshape(128, 1)
    gw1 = np.asarray(gW1).astype(NP_BF16)
    gw2 = np.asarray(gW2).astype(NP_BF16)
    gb1r = np.ascontiguousarray(np.broadcast_to(
        np.tile(np.asarray(gb1, np.float32) * (S_AGG / 1.0), BL), (128, 512)))
    gb2r = np.ascontiguousarray(np.broadcast_to(
        np.tile(np.asarray(gb2, np.float32), 2)[None, None, :], (128, 2, 256)))
    b1r = np.ascontiguousarray(np.broadcast_to(np.asarray(b1, np.float32), (BL, MLPD)))
    w2r = np.ascontiguousarray(np.broadcast_to(np.asarray(W2, np.float32)[:, 0], (BL, MLPD)))
    b2r = np.full((BL, 1), np.asarray(b2, np.float32)[0], np.float32)

    W1r = np.asarray(W1, np.float32).reshape(N, G2, MLPD)
    x_np = np.asarray(x, np.float32)

    in_maps = []
    for c in range(NCORES):
        xT = np.ascontiguousarray(
            x_np[c * BL : (c + 1) * BL].transpose(0, 2, 1)
        ).astype(NP_BF16)
        w1x = np.ascontiguousarray(
            W1r[c * NSH : (c + 1) * NSH].reshape(2, 128, G2, MLPD).transpose(1, 0, 2, 3)
        ).astype(NP_BF16)
        in_maps.append({
            "xT": xT, "at8": at8, "wc": wc, "cb": cb,
            "gw1": gw1, "gb1r": gb1r, "gw2": gw2, "gb2r": gb2r,
            "w1x": w1x, "b1r": b1r, "w2r": w2r, "b2r": b2r,
        })
    return in_maps


_NC_CACHE = {}


def kernel(**inputs) -> np.ndarray:
    key = "full"
    if key not in _NC_CACHE:
        _NC_CACHE[key] = build_nc()
    nc = _NC_CACHE[key]
    in_maps = _prep_inputs(**inputs)
    res = run_bass_kernel_spmd(nc, in_maps, core_ids=list(range(NCORES)))
    rows = np.concatenate([res.results[c]["out"] for c in range(NCORES)], axis=0)
    # row p of the permuted batch (p = s_local*8 + src_core) is global
    # sample b = src_core*BL + s_local
    out = np.empty_like(rows)
    for p in range(B):
        out[(p % NCORES) * BL + p // NCORES] = rows[p]
    return out.astype(np.float32)


# revision 10
# speedup vs baseline: 1.1558x; 1.0517x over previous
"""Trainium2 Bass kernel for CNN+GCN+MLP (nn_CNNGCN_18236431139458).

Strategy (8 NeuronCores, one chip), v2:
  - Data-parallel over batch (4 samples/core) for conv + both GCN layers.
  - The scatter-aggregate is a dense matmul against A^T (built host-side
    from edge_index), resident in SBUF as fp8 (x16 scale), shared by
    BOTH layers, run in DoubleRow fp8 (2x PE rate):
      layer 1: original order (linear then aggregate), as before.
      layer 2: swapped order  (aggregate h1 first, then linear) --
        h1 >= 0 post-relu, so the aggregation is a positive-weighted sum
        and fp8 quantization noise averages away (~1e-2 rel-l2 vs the
        2e-2 gate, validated host-side).
  - agg2 -> lin2 -> AllToAll pipelined per sample so the reshard hides
    under the next sample's compute.
  - MLP: W1 row-sharded over nodes across cores (fully SBUF-resident,
    loaded once); AllToAll delivers node-sharded h2; each core computes
    a partial [32, 100]; ReduceScatter sums and scatters; tiny local
    MLP tail.
  - DMA split: big streams (A^T, W1 shard) on GpSimd/SWDGE; input +
    staging + latency-critical transfers on Sync/HWDGE.

Layouts (per core):
  xT   [128 ic, 4 s, 2050]       feature-major input slices
  h0T  [128 oc, 4 s, 2048 n]     conv output (feature partitions)
  hw1p [128 n, 16 nch, 4sx128g]  fp8(64 * h0 @ gW1)   (node partitions)
  at8  [128 src, 16 chunk, 2048] fp8(16 * A^T)        (resident)
  h1p  [128 n, 16 nch, 4sx128g]  fp8(64 * relu(agg1 + b1))
  a2   [128 g, 4 s, 2048 dst]    bf16(A^T @ h1)       (feature partitions)
  h2   [128 n, 16 nt, 4 s, 128]  bf16 relu(a2 @ gW2 + b2) (node partitions)
  h2a  [128 n, 2 nt, 32 b, 128]  A2A landing (this core's 256-node shard)
  w1x  [128 n, 2 nt, 128 f, 100] W1 row-shard, reordered (resident)
"""

import numpy as np
import ml_dtypes

import concourse.bass as bass
import concourse.mybir as mybir
import concourse.tile as tile
from concourse.tile import add_dep_helper
from concourse import bacc
from concourse.bass_utils import run_bass_kernel_spmd

BF16 = mybir.dt.bfloat16
FP8 = mybir.dt.float8e4
F32 = mybir.dt.float32
NP_BF16 = ml_dtypes.bfloat16
NP_FP8 = mybir.dt.np(FP8)

B, H, E = 32, 2050, 128
N = 2048
C = 128
G1 = G2 = 128
MLPD = 100
KS = 3
NCORES = 8
BL = B // NCORES          # 4 samples per core
NSH = N // NCORES         # 256 nodes per core (W1 row shard)
RG = [list(range(NCORES))]

S_HW = 64.0               # fp8 scale for hw1 / h1
S_AT = 16.0               # fp8 scale for A^T
S_AGG = S_HW * S_AT       # psum scale after fp8 x fp8 aggregation

Relu = mybir.ActivationFunctionType.Relu
DoubleRow = mybir.MatmulPerfMode.DoubleRow


def build_nc(n_warm=120, collectives=True):
    nc = bacc.Bacc("TRN2", target_bir_lowering=False, debug=False,
                   num_devices=NCORES)

    d_xT = nc.dram_tensor("xT", [BL, 128, H], BF16, kind="ExternalInput").ap()
    d_at8 = nc.dram_tensor("at8", [16, 128, N], FP8, kind="ExternalInput").ap()
    d_wc = nc.dram_tensor("wc", [128, KS, 128], BF16, kind="ExternalInput").ap()
    d_cb = nc.dram_tensor("cb", [128, 1], F32, kind="ExternalInput").ap()
    d_gw1 = nc.dram_tensor("gw1", [128, 128], BF16, kind="ExternalInput").ap()
    d_gb1r = nc.dram_tensor("gb1r", [128, 512], F32, kind="ExternalInput").ap()
    d_gw2 = nc.dram_tensor("gw2", [128, 128], BF16, kind="ExternalInput").ap()
    d_gb2r = nc.dram_tensor("gb2r", [128, 2, 256], F32, kind="ExternalInput").ap()
    d_w1x = nc.dram_tensor("w1x", [128, 2, 128, MLPD], BF16, kind="ExternalInput").ap()
    d_b1r = nc.dram_tensor("b1r", [BL, MLPD], F32, kind="ExternalInput").ap()
    d_w2r = nc.dram_tensor("w2r", [BL, MLPD], F32, kind="ExternalInput").ap()
    d_b2r = nc.dram_tensor("b2r", [BL, 1], F32, kind="ExternalInput").ap()
    d_out = nc.dram_tensor("out", [BL, 1], F32, kind="ExternalOutput").ap()

    with tile.TileContext(nc) as tc:
        with (
            tc.tile_pool(name="const", bufs=1) as const,
            tc.tile_pool(name="big", bufs=1) as big,
            tc.tile_pool(name="tmp", bufs=3) as tmp,
            tc.tile_pool(name="small", bufs=1) as small,
            tc.tile_pool(name="psum", bufs=7, space="PSUM") as psum,
            tc.tile_pool(name="psum2", bufs=1, space="PSUM") as psum2,
            tc.tile_pool(name="dram", bufs=1, space="DRAM") as dram,
        ):
            # ---- SBUF loads ----
            # sync/HWDGE: conv weights first (conv stationary unblocks at
            # ~1us), then x, then later-needed smalls
            wc_sb = const.tile([128, KS, 128], BF16, tag="wc", name="wc_sb")
            nc.sync.dma_start(wc_sb[:], d_wc[:])
            cb_sb = const.tile([128, 1], F32, tag="cb", name="cb_sb")
            nc.sync.dma_start(cb_sb[:], d_cb[:])
            gw1_sb = const.tile([128, 128], BF16, tag="gw1", name="gw1_sb")
            nc.sync.dma_start(gw1_sb[:], d_gw1[:])
            xT_sb = big.tile([128, BL, H], BF16, tag="xt_a2", bufs=1, name="xT_sb")
            nc.sync.dma_start(xT_sb[:, 0, :], d_xT[0])
            nc.sync.dma_start(xT_sb[:, 1:4, :],
                              d_xT[1:4].rearrange("s p h -> p s h"))
            gw2_sb = const.tile([128, 128], BF16, tag="gw2", name="gw2_sb")
            nc.sync.dma_start(gw2_sb[:], d_gw2[:])
            gb1r_sb = const.tile([128, 512], F32, tag="gb1r", name="gb1r_sb")
            nc.sync.dma_start(gb1r_sb[:], d_gb1r[:])
            gb2r_sb = const.tile([128, 2, 256], F32, tag="gb2r", name="gb2r_sb")
            nc.sync.dma_start(gb2r_sb[:], d_gb2r[:])
            b1r_sb = small.tile([BL, MLPD], F32, tag="b1r", name="b1r_sb")
            nc.sync.dma_start(b1r_sb[:], d_b1r[:])
            w2r_sb = small.tile([BL, MLPD], F32, tag="w2r", name="w2r_sb")
            nc.sync.dma_start(w2r_sb[:], d_w2r[:])
            b2r_sb = small.tile([BL, 1], F32, tag="b2r", name="b2r_sb")
            nc.sync.dma_start(b2r_sb[:], d_b2r[:])

            # gpsimd/SWDGE: big resident streams (A^T fp8 first, then W1)
            at8_sb = const.tile([128, 16, N], FP8, tag="at8", name="at8_sb")
            at8_dmas = [
                nc.gpsimd.dma_start(
                    at8_sb[:, 4 * q : 4 * q + 4, :],
                    d_at8[4 * q : 4 * q + 4].rearrange("c p d -> p c d"),
                )
                for q in range(4)
            ]
            w1x_sb = const.tile([128, 2, 128, MLPD], BF16, tag="w1x", name="w1x_sb")
            w1x_dmas = [nc.gpsimd.dma_start(w1x_sb[:, q, :, :], d_w1x[:, q, :, :])
                        for q in range(2)]

            # ---- pre-conv PE heater: release the HAM clock gate before
            #      the real matmul stream starts ----
            heat_ps = psum.tile([128, 384], F32, tag="ps", name="heat_ps")
            for w in range(14):
                nc.tensor.matmul(
                    heat_ps[:],
                    lhsT=wc_sb[:, 0, :],
                    rhs=wc_sb[:],
                    start=(w == 0),
                    stop=(w == 13),
                )
            heat_sb = small.tile([128, 1], F32, tag="heat_sb", name="heat_sb")
            nc.vector.tensor_copy(heat_sb[:], heat_ps[:, 0:1])
            heat_dr = dram.tile([128, 1], F32, tag="heat_dr", name="heat_dr")
            nc.sync.dma_start(heat_dr[:], heat_sb[:])

            # ---- conv: h0T[oc, n] = relu(sum_k WcT_k.T @ xT[:, n+k] + cb),
            #      sample-outer so conv(s0) starts as soon as x(s0) lands ----
            h0T = big.tile([128, BL, N], BF16, tag="h0_h2a", bufs=1, name="h0T")
            for s in range(BL):
                for nt in range(4):
                    ps = psum.tile([128, 512], F32, tag="ps", name="ps_conv")
                    for k in range(KS):
                        nc.tensor.matmul(
                            ps[:],
                            lhsT=wc_sb[:, k, :],
                            rhs=xT_sb[:, s, nt * 512 + k : nt * 512 + k + 512],
                            start=(k == 0),
                            stop=(k == KS - 1),
                        )
                    act = nc.scalar.activation(h0T[:, s, nt * 512 : (nt + 1) * 512],
                                               ps[:], Relu, bias=cb_sb[:])
                    if s == 0 and nt == 0:
                        for dm in at8_dmas + w1x_dmas:
                            add_dep_helper(dm.ins, act.ins,
                                           reason="delay big loads past startup crunch")

            # ---- lin1: hw1p[n, (s,g)] = fp8(64 * h0 @ gW1) ----
            hw1p = big.tile([128, 16, 512], FP8, tag="hw1p", bufs=1, name="hw1p")
            for nch in range(16):
                ps = psum.tile([128, 512], F32, tag="ps", name="ps_lin1")
                for s in range(BL):
                    nc.tensor.matmul(
                        ps[:, s * 128 : (s + 1) * 128],
                        lhsT=h0T[:, s, nch * 128 : (nch + 1) * 128],
                        rhs=gw1_sb[:],
                        start=True,
                        stop=True,
                    )
                nc.vector.tensor_scalar_mul(hw1p[:, nch, :], ps[:], S_HW)

            # ---- agg1 (orientation-swapped, fp8 DoubleRow):
            #      out[dst, (s,g)] = sum_src at8[src, dst] * hw1p[src, (s,g)]
            #      h1p = fp8(relu((psum + 1024*b1row) / 16)) = fp8(64 * h1) ----
            h1p = big.tile([128, 16, 512], FP8, tag="h1p", bufs=1, name="h1p")
            for dt in range(16):
                ps = psum.tile([128, 512], F32, tag="ps", name="ps_agg1")
                for sc2 in range(8):
                    nc.tensor.matmul(
                        ps[:],
                        lhsT=at8_sb[:, 2 * sc2 : 2 * sc2 + 2, dt * 128 : (dt + 1) * 128],
                        rhs=hw1p[:, 2 * sc2 : 2 * sc2 + 2, :],
                        start=(sc2 == 0),
                        stop=(sc2 == 7),
                        perf_mode=DoubleRow,
                    )
                t1 = tmp.tile([128, 512], F32, tag="t1", bufs=3, name="t1")
                nc.vector.tensor_add(t1[:], ps[:], gb1r_sb[:])
                nc.scalar.activation(h1p[:, dt, :], t1[:], Relu, scale=1.0 / S_AT)

            # ---- per-sample: agg2 (swapped order, fp8 DR) -> lin2 -> A2A ----
            a2 = big.tile([128, BL, N], BF16, tag="xt_a2", bufs=1, name="a2")
            h2 = big.tile([128, BL, 8, 256], BF16, tag="h2", bufs=1, name="h2")
            # h2a: A2A landing, batch order permuted to (s_local, src_core)
            h2a = big.tile([128, BL, NCORES, 256], BF16, tag="h0_h2a", bufs=1,
                           name="h2a")
            a2a_bufs = []
            for s in range(BL):
                # agg2: a2[g, dst] = sum_src at8[src, dst] * h1p[src, (s,g)]
                for dt in range(4):
                    ps = psum.tile([128, 512], F32, tag="ps", name="ps_agg2")
                    for sc2 in range(8):
                        nc.tensor.matmul(
                            ps[:],
                            lhsT=h1p[:, 2 * sc2 : 2 * sc2 + 2, s * 128 : (s + 1) * 128],
                            rhs=at8_sb[:, 2 * sc2 : 2 * sc2 + 2, dt * 512 : (dt + 1) * 512],
                            start=(sc2 == 0),
                            stop=(sc2 == 7),
                            perf_mode=DoubleRow,
                        )
                    nc.vector.tensor_scalar_mul(a2[:, s, dt * 512 : (dt + 1) * 512],
                                                ps[:], 1.0 / S_AGG)
                # lin2: h2[dst, g2] = relu(a2[:, dst].T @ gW2 + b2row)
                for ntg in range(4):
                    ps2 = psum.tile([128, 2, 256], F32, tag="ps", name="ps_lin2")
                    for ntl in range(4):
                        nt = ntg * 4 + ntl
                        nc.tensor.matmul(
                            ps2[:, ntl // 2, (ntl % 2) * 128 : (ntl % 2) * 128 + 128],
                            lhsT=a2[:, s, nt * 128 : (nt + 1) * 128],
                            rhs=gw2_sb[:],
                            start=True,
                            stop=True,
                        )
                    t2 = tmp.tile([128, 2, 256], F32, tag="t2", bufs=3, name="t2")
                    nc.vector.tensor_add(t2[:], ps2[:], gb2r_sb[:])
                    nc.scalar.activation(h2[:, s, 2 * ntg : 2 * ntg + 2, :],
                                         t2[:], Relu)
                # A2A staging: stage each finished sample immediately; one
                # collective per PAIR of samples (fewer serialized cc ops)
                g = s // 2
                if s % 2 == 0:
                    a2a_bufs.append((
                        dram.tile([NCORES, 128, 2, 256], BF16, tag=f"a2a_in{g}",
                                  name=f"a2a_in{g}"),
                        dram.tile([NCORES, 128, 2, 256], BF16, tag=f"a2a_out{g}",
                                  name=f"a2a_out{g}"),
                    ))
                a2a_in, a2a_out = a2a_bufs[g]
                nc.sync.dma_start(a2a_in[:, :, s % 2, :].rearrange("j p m -> p j m"),
                                  h2[:, s, :, :])
                if s % 2 == 1 and collectives:
                    nc.gpsimd.collective_compute(
                        "AllToAll", mybir.AluOpType.bypass, replica_groups=RG,
                        ins=[a2a_in.opt()], outs=[a2a_out.opt()],
                    )

            # drain the A2A results (one DMA per sample)
            for s in range(BL):
                a2a_in, a2a_out = a2a_bufs[s // 2]
                srcbuf = a2a_out if collectives else a2a_in
                nc.sync.dma_start(h2a[:, s, :, :],
                                  srcbuf[:, :, s % 2, :].rearrange("i p m -> p i m"))

            # ---- PE warmers across the A2A wait (keep HAM at full clock) ----
            if n_warm > 0:
                warm_ps = psum.tile([128, 512], F32, tag="ps", name="warm_ps")
                for w in range(n_warm):
                    nc.tensor.matmul(
                        warm_ps[:],
                        lhsT=gw2_sb[:],
                        rhs=a2[:, BL - 1, 0:512],
                        start=(w == 0),
                        stop=(w == n_warm - 1),
                    )
                warm_sb = small.tile([128, 1], F32, tag="warm_sb", name="warm_sb")
                nc.vector.tensor_copy(warm_sb[:], warm_ps[:, 0:1])
                warm_dr = dram.tile([128, 1], F32, tag="warm_dr", name="warm_dr")
                nc.sync.dma_start(warm_dr[:], warm_sb[:])

            # ---- z[b, c] = sum_{nt,f} h2a[:, nt, :, f].T @ w1x[:, nt, f, :]
            #      4 col-groups (tile_position) keep the PE array full ----
            ps_z = psum2.tile([128, MLPD], F32, tag="psz", name="ps_z")
            for nt in range(2):
                for f in range(128):
                    k = nt * 128 + f
                    j = k % 4
                    nc.tensor.matmul(
                        ps_z[32 * j : 32 * (j + 1), :],
                        lhsT=h2a[:, :, :, nt * 128 + f],
                        rhs=w1x_sb[:, nt, f, :],
                        start=(k < 4),
                        stop=(k >= 252),
                        tile_position=(0, 32 * j),
                    )

            z_sb = small.tile([32, MLPD], F32, tag="z_sb", name="z_sb")
            nc.vector.tensor_copy(z_sb[:], ps_z[0:32, :])
            for j in range(1, 4):
                nc.vector.tensor_add(z_sb[:], z_sb[:], ps_z[32 * j : 32 * (j + 1), :])
            rs_in = dram.tile([32, MLPD], F32, tag="rs_in", name="rs_in")
            rs_out = dram.tile([BL, MLPD], F32, tag="rs_out", name="rs_out")
            nc.sync.dma_start(rs_in[:], z_sb[:])
            zloc = small.tile([BL, MLPD], F32, tag="zloc", name="zloc")
            if collectives:
                nc.gpsimd.collective_compute(
                    "ReduceScatter", mybir.AluOpType.add, replica_groups=RG,
                    ins=[rs_in.opt()], outs=[rs_out.opt()],
                )
                nc.sync.dma_start(zloc[:], rs_out[:])
            else:
                nc.sync.dma_start(zloc[:], rs_in[0:BL, :])
            hm = small.tile([BL, MLPD], F32, tag="hm", name="hm")
            nc.vector.tensor_add(hm[:], zloc[:], b1r_sb[:])
            nc.vector.tensor_scalar_max(hm[:], hm[:], 0.0)
            nc.vector.tensor_mul(hm[:], hm[:], w2r_sb[:])
            osb = small.tile([BL, 1], F32, tag="osb", name="osb")
            nc.vector.reduce_sum(osb[:], hm[:], axis=mybir.AxisListType.X)
            nc.vector.tensor_add(osb[:], osb[:], b2r_sb[:])
            nc.sync.dma_start(d_out[:], osb[:])

    nc.compile()
    return nc


def _prep_inputs(x, edge_index, conv_w, conv_b, gW1, gb1, gW2, gb2, W1, b1, W2, b2):
    """Host-side sharding / layout prep -> per-core input maps."""
    # gcn_norm (add_self_loops=True), duplicated edges accumulate
    src = np.concatenate([np.asarray(edge_index[0]), np.arange(N, dtype=np.int64)])
    dst = np.concatenate([np.asarray(edge_index[1]), np.arange(N, dtype=np.int64)])
    deg = np.bincount(dst, minlength=N).astype(np.float32)
    dinv = (1.0 / np.sqrt(np.maximum(deg, 1.0))).astype(np.float32)
    normv = dinv[src] * dinv[dst]
    AT = np.zeros((N, N), np.float32)
    np.add.at(AT, (src, dst), normv)
    at8 = np.ascontiguousarray((AT * S_AT).reshape(16, 128, N)).astype(NP_FP8)

    wc = np.ascontiguousarray(
        np.asarray(conv_w)[:, 0, :, :].transpose(2, 1, 0)  # [ic, KS, oc]
    ).astype(NP_BF16)
    cb = np.asarray(conv_b, np.float32).reshape(128, 1)
    gw1 = np.asarray(gW1).astype(NP_BF16)
    gw2 = np.asarray(gW2).astype(NP_BF16)
    gb1r = np.ascontiguousarray(np.broadcast_to(
        np.tile(np.asarray(gb1, np.float32) * (S_AGG / 1.0), BL), (128, 512)))
    gb2r = np.ascontiguousarray(np.broadcast_to(
        np.tile(np.asarray(gb2, np.float32), 2)[None, None, :], (128, 2, 256)))
    b1r = np.ascontiguousarray(np.broadcast_to(np.asarray(b1, np.float32), (BL, MLPD)))
    w2r = np.ascontiguousarray(np.broadcast_to(np.asarray(W2, np.float32)[:, 0], (BL, MLPD)))
    b2r = np.full((BL, 1), np.asarray(b2, np.float32)[0], np.float32)

    W1r = np.asarray(W1, np.float32).reshape(N, G2, MLPD)
    x_np = np.asarray(x, np.float32)

    in_maps = []
    for c in range(NCORES):
        xT = np.ascontiguousarray(
            x_np[c * BL : (c + 1) * BL].transpose(0, 2, 1)
        ).astype(NP_BF16)
        w1x = np.ascontiguousarray(
            W1r[c * NSH : (c + 1) * NSH].reshape(2, 128, G2, MLPD).transpose(1, 0, 2, 3)
        ).astype(NP_BF16)
        in_maps.append({
            "xT": xT, "at8": at8, "wc": wc, "cb": cb,
            "gw1": gw1, "gb1r": gb1r, "gw2": gw2, "gb2r": gb2r,
            "w1x": w1x, "b1r": b1r, "w2r": w2r, "b2r": b2r,
        })
    return in_maps


_NC_CACHE = {}


def kernel(**inputs) -> np.ndarray:
    key = "full"
    if key not in _NC_CACHE:
        _NC_CACHE[key] = build_nc()
    nc = _NC_CACHE[key]
    in_maps = _prep_inputs(**inputs)
    res = run_bass_kernel_spmd(nc, in_maps, core_ids=list(range(NCORES)))
    rows = np.concatenate([res.results[c]["out"] for c in range(NCORES)], axis=0)
    # row p of the permuted batch (p = s_local*8 + src_core) is global
    # sample b = src_core*BL + s_local
    out = np.empty_like(rows)
    for p in range(B):
        out[(p % NCORES) * BL + p // NCORES] = rows[p]
    return out.astype(np.float32)
